# revision 1
# baseline (speedup 1.0000x reference)
"""Trainium2 Bass kernel for nn_CrossAttention_37220186587177.

Cross-attention: B=2, L=S=2048, D=1024, H=16 heads, Dh=64, RoPE on q/k,
softmax over S, out-projection.

Sharding (8 NeuronCores): data-parallel over B (2 groups of 4 cores),
tensor-parallel over heads within a group (4 heads/core).  Each core
computes its 4 heads end-to-end plus a partial out-projection over its
256 contraction dims; the 4 partial [L, D] outputs per batch are summed
on the host (cheaper than an on-device collective for this size), and
bo is added there.

Device-side layout choices:
 - Activations are shipped pre-transposed (xT/eT = [D, seq]) so the
   contraction dim lands on SBUF partitions with contiguous DMA.
 - RoPE: weights' output columns are permuted on the host so each head's
   even/odd rotary lanes form two separate 32-partition blocks ("e" and
   "o" tiles).  RoPE then becomes lane-aligned elementwise ops against
   replicated cos/sin tables, and scores are computed as two K=32
   accumulating matmuls per head (row-packed across heads).
 - Scores are computed transposed (scoresT[s, l]) so the PV matmul needs
   no transposes; softmax denominators come for free from a ones-column
   appended to V (M=65 PV matmuls); the division is done via a K=1
   broadcast matmul of the reciprocal row.
 - All matmuls run in float32r (1 cycle/row like bf16 for free dim >=256,
   but ~16x better accuracy: measured 1.5e-4 vs 2.4e-3 fro error).
 - key_padding_mask is applied by zeroing masked rows of V (including the
   ones-column), which is exactly equivalent to -inf score masking.
"""
import sys

if "/opt/trn_rl_repo" not in sys.path:
    sys.path.insert(0, "/opt/trn_rl_repo")

import numpy as np

import concourse.bacc as bacc
import concourse.mybir as mybir
import concourse.tile as tile
from concourse import bass_utils
from concourse.bass import ts

B, L, S, D, H, Dh = 2, 2048, 2048, 1024, 16, 64
NCORES = 8
HPC = 4              # heads per core
Dc = HPC * Dh        # 256 per-core head dims
F32 = mybir.dt.float32
F32R = mybir.dt.float32r
AF = mybir.ActivationFunctionType
SCALE = Dh ** -0.5   # 0.125


def build_nc(mm_dtype="f32r"):
    DT = {"f32r": F32R, "bf16": mybir.dt.bfloat16, "f32": F32}[mm_dtype]
    nc = bacc.Bacc("TRN2", target_bir_lowering=False, debug=False)

    xT = nc.dram_tensor("xT", [D, L], DT, kind="ExternalInput")
    eT = nc.dram_tensor("eT", [D, S], DT, kind="ExternalInput")
    wq = nc.dram_tensor("wq", [D, Dc], DT, kind="ExternalInput")
    wk = nc.dram_tensor("wk", [D, Dc], DT, kind="ExternalInput")
    wv = nc.dram_tensor("wv", [D, HPC * (Dh + 1)], DT, kind="ExternalInput")
    wo = nc.dram_tensor("wo", [Dc, D], DT, kind="ExternalInput")
    qkb = nc.dram_tensor("qkb", [128, 4], F32, kind="ExternalInput")
    smalls = nc.dram_tensor("smalls", [1, 512], DT, kind="ExternalInput")
    cost = nc.dram_tensor("cost", [128, S], DT, kind="ExternalInput")
    sint = nc.dram_tensor("sint", [128, S], DT, kind="ExternalInput")
    vmask = nc.dram_tensor("vmask", [128, 16], F32, kind="ExternalInput")
    y = nc.dram_tensor("y", [L, D], F32, kind="ExternalOutput")

    with tile.TileContext(nc) as tc:
        with tc.tile_pool(name="const", bufs=1) as cpool, \
             tc.tile_pool(name="actin", bufs=10) as apool, \
             tc.tile_pool(name="qk", bufs=1) as qkpool, \
             tc.tile_pool(name="tmp", bufs=3) as tpool, \
             tc.tile_pool(name="vsb", bufs=1) as vpool, \
             tc.tile_pool(name="ex", bufs=3) as epool, \
             tc.tile_pool(name="on", bufs=3) as onpool, \
             tc.tile_pool(name="bc", bufs=2) as bcpool, \
             tc.tile_pool(name="rc", bufs=4) as rcpool, \
             tc.tile_pool(name="yo", bufs=3) as ypool, \
             tc.tile_pool(name="scp", bufs=2, space="PSUM") as scp, \
             tc.tile_pool(name="pvp", bufs=4, space="PSUM") as pvp:

            # ---- constants ----
            w_q = cpool.tile([128, 8, Dc], DT, name="w_q")
            w_k = cpool.tile([128, 8, Dc], DT, name="w_k")
            w_v = cpool.tile([128, 8, HPC * (Dh + 1)], DT, name="w_v")
            w_o = cpool.tile([128, 2, D], DT, name="w_o")
            nc.sync.dma_start(out=w_q[:], in_=wq.ap().rearrange("(a p) m -> p a m", p=128))
            nc.sync.dma_start(out=w_k[:], in_=wk.ap().rearrange("(a p) m -> p a m", p=128))
            nc.sync.dma_start(out=w_v[:], in_=wv.ap().rearrange("(a p) m -> p a m", p=128))
            nc.sync.dma_start(out=w_o[:], in_=wo.ap().rearrange("(a p) m -> p a m", p=128))
            qkb_t = cpool.tile([128, 4], F32, name="qkb_t")
            nc.sync.dma_start(out=qkb_t[:], in_=qkb.ap())
            sm_t = cpool.tile([1, 512], DT, name="sm_t")
            nc.sync.dma_start(out=sm_t[:], in_=smalls.ap())
            cost_t = cpool.tile([128, S], DT, name="cost_t")
            sint_t = cpool.tile([128, S], DT, name="sint_t")
            nc.sync.dma_start(out=cost_t[:], in_=cost.ap())
            nc.sync.dma_start(out=sint_t[:], in_=sint.ap())
            vmask_t = cpool.tile([128, 16], F32, name="vmask_t")
            nc.sync.dma_start(out=vmask_t[:], in_=vmask.ap())

            # ---- phase 1a: Q projection (qT in e/o-split layout) + bias ----
            qe = qkpool.tile([128, L], DT, name="qe", tag="qe")
            qo = qkpool.tile([128, L], DT, name="qo", tag="qo")
            ke = qkpool.tile([128, S], DT, name="ke", tag="ke")
            ko = qkpool.tile([128, S], DT, name="ko", tag="ko")
            q_eo = (qe, qo)
            k_eo = (ke, ko)

            for lc in range(4):
                x_tiles = []
                for d in range(8):
                    t = apool.tile([128, 512], DT, name=f"x_{d}_{lc}", tag="act")
                    nc.sync.dma_start(out=t[:], in_=xT.ap()[ts(d, 128), ts(lc, 512)])
                    x_tiles.append(t)
                for m in range(2):
                    ps = scp.tile([128, 512], F32, name=f"qps_{m}_{lc}", tag="sc")
                    for d in range(8):
                        nc.tensor.matmul(ps[:], w_q[:, d, ts(m, 128)], x_tiles[d][:],
                                         start=(d == 0), stop=(d == 7))
                    nc.vector.tensor_scalar_add(q_eo[m][:, ts(lc, 512)], ps[:],
                                                qkb_t[:, m:m + 1])

            # ---- phase 1b: K projection + V projection ----
            v_tiles = []
            for sc in range(4):
                e_tiles = []
                for d in range(8):
                    t = apool.tile([128, 512], DT, name=f"e_{d}_{sc}", tag="act")
                    nc.sync.dma_start(out=t[:], in_=eT.ap()[ts(d, 128), ts(sc, 512)])
                    e_tiles.append(t)
                for m in range(2):
                    ps = scp.tile([128, 512], F32, name=f"kps_{m}_{sc}", tag="sc")
                    for d in range(8):
                        nc.tensor.matmul(ps[:], w_k[:, d, ts(m, 128)], e_tiles[d][:],
                                         start=(d == 0), stop=(d == 7))
                    nc.vector.tensor_scalar_add(k_eo[m][:, ts(sc, 512)], ps[:],
                                                qkb_t[:, m + 2:m + 3])
                for sb in range(4):
                    s_blk = 4 * sc + sb
                    wid = HPC * (Dh + 1)  # 260: per head [v(64) | ones-col]
                    ps = scp.tile([128, wid], F32, name=f"vps_{s_blk}", tag="sc")
                    # bias+ones first (start=True): v-cols get bv, 65th col gets 1.0
                    nc.tensor.matmul(ps[:], sm_t[0:1, 320:448], sm_t[0:1, 0:wid],
                                     start=True, stop=False)
                    for d in range(8):
                        nc.tensor.matmul(ps[:], e_tiles[d][:, ts(sb, 128)], w_v[:, d, :],
                                         start=False, stop=(d == 7))
                    vt = vpool.tile([128, wid], DT, name=f"v_{s_blk}",
                                    tag=f"v{s_blk}")
                    # mask fold: vt = ps * mask[s] (zeroes masked V rows incl ones col)
                    nc.vector.tensor_scalar_mul(vt[:], ps[:],
                                                vmask_t[:, s_blk:s_blk + 1])
                    v_tiles.append(vt)

            # ---- RoPE on q and k (in place) ----
            for (pe, po) in (q_eo, k_eo):
                t1 = tpool.tile([128, L], DT, name=f"t1_{pe.tensor.name}", tag="tmp")
                t2 = tpool.tile([128, L], DT, name=f"t2_{pe.tensor.name}", tag="tmp")
                t3 = tpool.tile([128, L], DT, name=f"t3_{pe.tensor.name}", tag="tmp")
                nc.vector.tensor_mul(t1[:], pe[:], cost_t[:])
                nc.vector.tensor_mul(t2[:], pe[:], sint_t[:])
                nc.vector.tensor_mul(t3[:], po[:], sint_t[:])
                nc.vector.tensor_sub(pe[:], t1[:], t3[:])
                t4 = tpool.tile([128, L], DT, name=f"t4_{pe.tensor.name}", tag="tmp")
                nc.vector.tensor_mul(t4[:], po[:], cost_t[:])
                nc.vector.tensor_add(po[:], t4[:], t2[:])

            # ---- phase 2: attention + out-projection, per 512-wide l-chunk ----
            for lc in range(4):
                on_tiles = []
                for pr in range(2):
                    heads = (2 * pr, 2 * pr + 1)
                    pv = {}
                    for h in heads:
                        pv[h] = pvp.tile([Dh + 1, 512], F32, name=f"pv_{lc}_{h}",
                                         tag="pv")
                    for g in range(8):
                        ex = {}
                        for h in heads:
                            sc_ps = scp.tile([128, 1024], F32,
                                             name=f"sc_{lc}_{h}_{g}", tag="sc")
                            for j in range(2):
                                s_blk = 2 * g + j
                                ksl = slice(128 * s_blk, 128 * s_blk + 128)
                                lsl = slice(512 * lc, 512 * lc + 512)
                                hsl = slice(32 * h, 32 * h + 32)
                                nc.tensor.matmul(sc_ps[:, ts(j, 512)],
                                                 ke[hsl, ksl], qe[hsl, lsl],
                                                 start=True, stop=False,
                                                 tile_position=(32 * h, 0))
                                nc.tensor.matmul(sc_ps[:, ts(j, 512)],
                                                 ko[hsl, ksl], qo[hsl, lsl],
                                                 start=False, stop=True,
                                                 tile_position=(32 * h, 0))
                            ex[h] = epool.tile([128, 1024], DT,
                                               name=f"ex_{lc}_{h}_{g}", tag="ex")
                            nc.scalar.activation(ex[h][:], sc_ps[:], AF.Exp,
                                                 scale=SCALE)
                        for h in heads:
                            for j in range(2):
                                s_blk = 2 * g + j
                                nc.tensor.matmul(pv[h][:],
                                                 v_tiles[s_blk][:, ts(h, Dh + 1)],
                                                 ex[h][:, ts(j, 512)],
                                                 start=(s_blk == 0),
                                                 stop=(s_blk == 15))
                    on = onpool.tile([128, 512], DT, name=f"on_{lc}_{pr}", tag="on")
                    for idx, h in enumerate(heads):
                        rc = rcpool.tile([1, 512], DT, name=f"rc_{lc}_{h}", tag="rc")
                        with nc.allow_low_precision(reason="f32r recip for bcast mm"):
                            nc.vector.reciprocal(rc[:], pv[h][Dh:Dh + 1, :])
                        bc_ps = scp.tile([64, 512], F32, name=f"bc_{lc}_{h}", tag="sc")
                        nc.tensor.matmul(bc_ps[:], sm_t[0:1, 320:384], rc[:],
                                         start=True, stop=True)
                        bc_sb = bcpool.tile([64, 512], DT, name=f"bcs_{lc}_{h}",
                                            tag="bc")
                        nc.vector.tensor_copy(bc_sb[:], bc_ps[:])
                        nc.vector.tensor_mul(on[64 * idx:64 * (idx + 1), :],
                                             pv[h][0:Dh, :], bc_sb[:])
                    on_tiles.append(on)
                for lm in range(4):
                    for jb in range(2):
                        yps = scp.tile([128, 512], F32, name=f"yps_{lc}_{lm}_{jb}",
                                       tag="sc")
                        for pr in range(2):
                            nc.tensor.matmul(yps[:],
                                             on_tiles[pr][:, ts(lm, 128)],
                                             w_o[:, pr, ts(jb, 512)],
                                             start=(pr == 0), stop=(pr == 1))
                        ysb = ypool.tile([128, 512], F32, name=f"ysb_{lc}_{lm}_{jb}",
                                         tag="y")
                        nc.vector.tensor_copy(ysb[:], yps[:])
                        nc.sync.dma_start(
                            out=y.ap()[512 * lc + 128 * lm:512 * lc + 128 * lm + 128,
                                       ts(jb, 512)],
                            in_=ysb[:])

    nc.compile()
    return nc


def make_in_maps(x, encoder_inputs, key_padding_mask, Wq, bq, Wk, bk, Wv, bv, Wo,
                 mm_dtype="f32r"):
    f32 = np.float32
    if mm_dtype == "bf16":
        import ml_dtypes
        mmdt = ml_dtypes.bfloat16
    else:
        mmdt = np.float32
    x = np.asarray(x, dtype=f32)
    enc = np.asarray(encoder_inputs, dtype=f32)
    mask = np.asarray(key_padding_mask)
    Wq = np.asarray(Wq, dtype=f32); bq = np.asarray(bq, dtype=f32)
    Wk = np.asarray(Wk, dtype=f32); bk = np.asarray(bk, dtype=f32)
    Wv = np.asarray(Wv, dtype=f32); bv = np.asarray(bv, dtype=f32)
    Wo = np.asarray(Wo, dtype=f32)

    inv_freq = (1.0 / (10000.0 ** (np.arange(0, Dh, 2, dtype=f32) / f32(Dh)))).astype(f32)
    ang = np.arange(S, dtype=f32)[:, None] * inv_freq[None, :]   # [S, 32]
    costab = np.tile(np.ascontiguousarray(np.cos(ang).T), (4, 1)).astype(f32)  # [128,S]
    sintab = np.tile(np.ascontiguousarray(np.sin(ang).T), (4, 1)).astype(f32)

    xTb = [np.ascontiguousarray(x[b].T) for b in range(B)]
    eTb = [np.ascontiguousarray(enc[b].T) for b in range(B)]
    maskb = [np.ascontiguousarray(mask[b].astype(f32).reshape(16, 128).T)
             for b in range(B)]

    in_maps = []
    for core in range(NCORES):
        b = core // 4
        heads = [(core % 4) * HPC + i for i in range(HPC)]
        eidx = np.concatenate([64 * h + np.arange(0, 64, 2) for h in heads])
        oidx = np.concatenate([64 * h + np.arange(1, 64, 2) for h in heads])
        eo = np.concatenate([eidx, oidx])
        nat = np.concatenate([64 * h + np.arange(64) for h in heads])

        qkb = np.stack([bq[eidx], bq[oidx], bk[eidx], bk[oidx]], axis=1)
        qkb = np.ascontiguousarray(qkb.astype(f32))
        # smalls: [0:260] = per-head [bv_h(64) | 1.0]; [320:448] = 1.0
        smalls = np.zeros((1, 512), f32)
        bv_pad = np.zeros((HPC, Dh + 1), f32)
        bv_pad[:, :Dh] = bv[nat].reshape(HPC, Dh)
        bv_pad[:, Dh] = 1.0
        smalls[0, :HPC * (Dh + 1)] = bv_pad.reshape(-1)
        smalls[0, 320:448] = 1.0
        # wv padded: per head 64 cols of Wv.T + one zero col
        wv_pad = np.zeros((D, HPC * (Dh + 1)), f32)
        wvT = Wv[nat, :].T.reshape(D, HPC, Dh)
        for h in range(HPC):
            wv_pad[:, h * (Dh + 1):h * (Dh + 1) + Dh] = wvT[:, h, :]

        in_maps.append({
            "xT": xTb[b].astype(mmdt),
            "eT": eTb[b].astype(mmdt),
            "wq": np.ascontiguousarray(Wq[eo, :].T).astype(mmdt),
            "wk": np.ascontiguousarray(Wk[eo, :].T).astype(mmdt),
            "wv": wv_pad.astype(mmdt),
            "wo": np.ascontiguousarray(Wo[:, nat].T).astype(mmdt),
            "qkb": qkb,
            "smalls": smalls.astype(mmdt),
            "cost": costab.astype(mmdt),
            "sint": sintab.astype(mmdt),
            "vmask": maskb[b],
        })
    return in_maps


_CACHE = {}


def _get_nc():
    if "nc" not in _CACHE:
        _CACHE["nc"] = build_nc()
    return _CACHE["nc"]


def kernel(x, encoder_inputs, key_padding_mask, Wq, bq, Wk, bk, Wv, bv, Wo, bo,
           _results_hook=None):
    nc = _get_nc()
    in_maps = make_in_maps(x, encoder_inputs, key_padding_mask,
                           Wq, bq, Wk, bk, Wv, bv, Wo)
    res = bass_utils.run_bass_kernel_spmd(nc, in_maps, list(range(NCORES)))
    if _results_hook is not None:
        _results_hook(res)
    bo = np.asarray(bo, dtype=np.float32)
    out = np.empty((B, L, D), np.float32)
    for b in range(B):
        acc = res.results[4 * b]["y"].astype(np.float32).copy()
        for c in range(4 * b + 1, 4 * b + 4):
            acc += res.results[c]["y"]
        out[b] = acc + bo[None, :]
    return out

